# revision 29
# baseline (speedup 1.0000x reference)
"""KAN layer kernel for 8 Trainium2 NeuronCores.

Math (reference):
    basis[b,i] = sum_h silu(x[b,i]*w1[i%K,h] + b1[i%K,h]) * w2[i%K,h] + b2[i%K]
    out[b,o]   = sum_i basis[b,i] * Wsum[o,i],   Wsum = W.sum(-1)   # [O,I]

Sharding: data-parallel over the input-feature axis I (16384 -> 8 x 2048).
Each core computes a partial out[64,1024] over its feature slice; host sums.

Per-core device program (memory-bound on reading its W slice):
  - W is host-cast to bf16 (tolerance 2e-2 >> bf16 error ~4e-3), halving
    HBM traffic vs fp32. Layout Wt[i, (k,o)]: 17 plain HWDGE loads of
    [128, 5120] stream on the Sync queue (10KB descriptors), consumed
    tile-by-tile by the PE; the final tile arrives in two k-slices so its
    matmuls overlap the DMA tail.
  - The k-reduction rides the PE: out[b,o] = sum_{i,k} basis[b,i]*W[o,i,k];
    each i-tile issues 5 (k) x 2 (O-half) matmuls reusing the same
    lhsT = basisT tile, accumulating all 80 into each PSUM bank.
  - basis is computed with i on partitions. DVE's bf16 2x mode needs every
    operand's last AP dim packed (stride 1): ACT materializes x broadcast
    over h (Copy), then all DVE tensor_tensor ops qualify. The h-reduction
    is a 2x-eligible add-tree; the last fold fuses +b2 and the bf16 cast in
    one scalar_tensor_tensor. ACT table thrash is avoided by pinning the
    silu_and_others table (serves Copy too) with a leading dummy silu.
  - w1/b1/w2/b2 patterns repeat every 5 i-tiles (128 % 5 = 3), so only 5
    pattern vectors ship instead of 16 per-tile copies.
"""
import numpy as np

B, I, O, K, H = 64, 16384, 1024, 5, 16
NCORES = 8
IC = I // NCORES          # 2048 features per core
P = 128                   # partition tile
NT = IC // P              # 16 i-tiles per core
NB = B                    # 64
NO = O                    # 1024
ROW = K * NO              # 5120 bf16 per Wt row
NSUB = 4                  # i-tiles per W chunk
NCH = NT // NSUB          # 4 chunks
# cb16 (bf16) column blocks: xs [NT*NB] | w1 [5*H] | b1 [5*H] | w2 [5*H]
X0 = NT * NB
X1 = X0 + 5 * H
X2 = X1 + 5 * H
CBW = X2 + 5 * H

TRACE = False             # test.py sets True to capture an NTFF profile
LAST_RESULT = None


def _build():
    from contextlib import ExitStack
    from concourse import bacc, mybir, tile

    f32 = mybir.dt.float32
    bf16 = mybir.dt.bfloat16
    nc = bacc.Bacc("TRN2", target_bir_lowering=False, debug=False,
                   num_devices=NCORES)
    f8 = mybir.dt.float8e4
    Wt = nc.declare_dram_parameter("Wt", [IC, 3 * NO], bf16, isOutput=False)
    Wt8 = nc.declare_dram_parameter("Wt8", [NCH, P, NSUB * 2 * NO], f8,
                                    isOutput=False)
    cb16d = nc.declare_dram_parameter("cb16", [P, CBW], bf16, isOutput=False)
    cb32d = nc.declare_dram_parameter("cb32", [P, 5], f32, isOutput=False)
    out = nc.declare_dram_parameter("out", [NB, NO], f32, isOutput=True)

    with tile.TileContext(nc) as tc, ExitStack() as ctx:
        const = ctx.enter_context(tc.tile_pool(name="const", bufs=1))
        wpool = ctx.enter_context(tc.tile_pool(name="w", bufs=1))
        ppool = ctx.enter_context(tc.tile_pool(name="pre", bufs=1))
        spool = ctx.enter_context(tc.tile_pool(name="silu", bufs=1))
        mpool = ctx.enter_context(tc.tile_pool(name="msum", bufs=1))
        apool = ctx.enter_context(tc.tile_pool(name="acc", bufs=NT))
        opool = ctx.enter_context(tc.tile_pool(name="out", bufs=1))
        psum = ctx.enter_context(tc.tile_pool(name="psum", bufs=1, space="PSUM"))

        # All bulk DMAs ride the Sync HWDGE queue: the Activation queue's
        # descriptor generation shares the ACT sequencer with Copy/Silu
        # compute and stalls badly. cb loads lead so compute starts right
        # after the fixed ~7.5us preamble; W streams tile-by-tile behind,
        # the last tile in two k-slices so its matmuls overlap the tail.
        cb16 = const.tile([P, CBW], bf16)
        cb32 = const.tile([P, 5], f32)
        nc.sync.dma_start(cb16[:, :], cb16d[:, :])
        nc.sync.dma_start(cb32[:, :], cb32d[:, :])

        ps0 = psum.tile([NB, 512], f32, tag="ps0")
        ps1 = psum.tile([NB, 512], f32, tag="ps1")

        # k=0 ships as fp8e4m3 (error headroom allows one of five slices),
        # chunk-contiguous so descriptors stay 4KB; k=1..4 stay bf16 and
        # stream tile-by-tile behind their chunk's fp8 load.
        wts, w8s = [], []
        for t in range(NT):
            if t % NSUB == 0:
                q = t // NSUB
                w8 = wpool.tile([P, NSUB * 2 * NO], f8, tag="w8", bufs=NCH)
                nc.sync.dma_start(w8[:, :], Wt8[q, :, :])
                w8s.append(w8)
            wt = wpool.tile([P, 3 * NO], bf16, tag="wt", bufs=6)
            if t == NT - 1:
                nc.sync.dma_start(wt[:, 0:2 * NO],
                                  Wt[t * P:(t + 1) * P, 0:2 * NO])
                nc.sync.dma_start(wt[:, 2 * NO:3 * NO],
                                  Wt[t * P:(t + 1) * P, 2 * NO:3 * NO])
            else:
                nc.sync.dma_start(wt[:, :], Wt[t * P:(t + 1) * P, :])
            wts.append(wt)

        def wk(t, k, lo, hi):
            # rhs slice for (tile, k): k=0,1 from the fp8 chunk, else bf16.
            if k < 2:
                q, s = divmod(t, NSUB)
                base = s * 2 * NO + k * NO
                return w8s[q][:, base + lo:base + hi]
            return wts[t][:, (k - 2) * NO + lo:(k - 2) * NO + hi]

        # Pin the silu_and_others table (it also serves Copy) so the whole
        # kernel needs one ACT_TABLE_LOAD.
        dummy = const.tile([1, 1], f32)
        nc.scalar.activation(dummy[:, :], cb32[0:1, 0:1],
                             mybir.ActivationFunctionType.Silu)

        xreps, pre2s, accs = [], [], []

        def emit_front(t):
            # ACT: materialize x broadcast over h; DVE: 2x-mode affine.
            g = (3 * t) % 5
            xs = cb16[:, t * NB:(t + 1) * NB]              # [P, 64]
            w1s = cb16[:, X0 + g * H:X0 + (g + 1) * H]     # [P, 16]
            b1s = cb16[:, X1 + g * H:X1 + (g + 1) * H]
            xrep = ppool.tile([P, NB, H], bf16, tag="xrep", bufs=3)
            nc.scalar.copy(xrep[:, :, :],
                           xs[:, :, None].to_broadcast([P, NB, H]))
            xreps.append(xrep)
            pre = ppool.tile([P, NB, H], bf16, tag="pre", bufs=2)
            nc.vector.tensor_tensor(
                pre[:, :, :], xrep[:, :, :],
                w1s[:, None, :].to_broadcast([P, NB, H]),
                mybir.AluOpType.mult)
            pre2 = ppool.tile([P, NB, H], bf16, tag="pre2", bufs=3)
            nc.vector.tensor_tensor(
                pre2[:, :, :], pre[:, :, :],
                b1s[:, None, :].to_broadcast([P, NB, H]),
                mybir.AluOpType.add)
            pre2s.append(pre2)

        def emit_back(t):
            # ACT: silu; DVE: w2 mult + 2x add-tree + fused +b2/bf16 cast.
            g = (3 * t) % 5
            w2s = cb16[:, X2 + g * H:X2 + (g + 1) * H]
            s = spool.tile([P, NB, H], bf16, tag="s", bufs=3)
            nc.scalar.activation(s[:, :, :], pre2s[t][:, :, :],
                                 mybir.ActivationFunctionType.Silu)
            sw = spool.tile([P, NB, H], bf16, tag="sw", bufs=2)
            nc.vector.tensor_tensor(
                sw[:, :, :], s[:, :, :],
                w2s[:, None, :].to_broadcast([P, NB, H]),
                mybir.AluOpType.mult)
            f1 = mpool.tile([P, NB, H // 2], bf16, tag="f1", bufs=2)
            nc.vector.tensor_tensor(
                f1[:, :, :], sw[:, :, 0:H // 2], sw[:, :, H // 2:H],
                mybir.AluOpType.add)
            f2 = mpool.tile([P, NB, H // 4], bf16, tag="f2", bufs=2)
            nc.vector.tensor_tensor(
                f2[:, :, :], f1[:, :, 0:H // 4], f1[:, :, H // 4:H // 2],
                mybir.AluOpType.add)
            f3 = mpool.tile([P, NB, 2], bf16, tag="f3", bufs=2)
            nc.vector.tensor_tensor(
                f3[:, :, :], f2[:, :, 0:2], f2[:, :, 2:4],
                mybir.AluOpType.add)
            acc = apool.tile([P, NB], bf16, tag="acc")
            # acc = (f3[...,0] + b2) + f3[...,1], cast to bf16
            nc.vector.scalar_tensor_tensor(
                acc[:, :], f3[:, :, 0], cb32[:, g:g + 1], f3[:, :, 1],
                op0=mybir.AluOpType.add, op1=mybir.AluOpType.add)
            accs.append(acc)

        # Software-pipelined emission: the front half (copy + affine) of
        # tile t+1 is emitted before the back half of tile t, so neither
        # engine ever waits on the other's just-issued work.
        emit_front(0)
        for t in range(1, NT):
            emit_front(t)
            emit_back(t - 1)
        emit_back(NT - 1)

        # ---- partial matmuls: out[b,o] += sum_k basisT.T @ W[:,k,:] ----
        # Last tile runs all ps0 matmuls before ps1's so the ps0 bank can
        # drain (copy + store) while ps1 is still accumulating.
        for t in range(NT - 1):
            for k in range(K):
                first = (t == 0 and k == 0)
                nc.tensor.matmul(ps0[:, :], accs[t][:, :],
                                 wk(t, k, 0, 512),
                                 start=first, stop=False)
                nc.tensor.matmul(ps1[:, :], accs[t][:, :],
                                 wk(t, k, 512, NO),
                                 start=first, stop=False)
        tl = NT - 1
        out_sb = opool.tile([NB, NO], f32)
        for k in range(K):
            nc.tensor.matmul(ps0[:, :], accs[tl][:, :],
                             wk(tl, k, 0, 512),
                             start=False, stop=(k == K - 1))
        nc.vector.tensor_copy(out_sb[:, 0:512], ps0[:, :])
        nc.sync.dma_start(out[:, 0:512], out_sb[:, 0:512])
        for k in range(K):
            nc.tensor.matmul(ps1[:, :], accs[tl][:, :],
                             wk(tl, k, 512, NO),
                             start=False, stop=(k == K - 1))
        nc.vector.tensor_copy(out_sb[:, 512:1024], ps1[:, :])
        nc.sync.dma_start(out[:, 512:1024], out_sb[:, 512:1024])
    nc.compile()
    return nc


def kernel(x, w1, b1, w2, b2, W):
    global LAST_RESULT
    import ml_dtypes
    from concourse.bass_utils import run_bass_kernel_spmd

    bf16 = ml_dtypes.bfloat16
    x = np.asarray(x, dtype=np.float32)
    W = np.asarray(W, dtype=np.float32)
    w1 = np.asarray(w1, dtype=np.float32)
    b1 = np.asarray(b1, dtype=np.float32)
    w2 = np.asarray(w2, dtype=np.float32)
    b2 = np.asarray(b2, dtype=np.float32)

    # ---- host prep: k=1..4 -> bf16 rows [i, (k,o)]; k=0 -> fp8e4m3,
    # chunk-contiguous [chunk, p, (sub,o)] ----
    f8 = ml_dtypes.float8_e4m3fn
    Wb = W.astype(bf16).view(np.uint16)                    # [O, I, K]
    Wtr = np.ascontiguousarray(Wb.transpose(1, 2, 0))      # [I, K, O] u16
    Wt16_full = np.ascontiguousarray(Wtr[:, 2:, :]).reshape(I, 3 * NO)
    W8_full = np.ascontiguousarray(
        W[:, :, 0:2].transpose(1, 2, 0)).astype(f8)        # [I, 2, O]

    pidx = np.arange(P)

    def patterns(shift):
        # Device uses g=(3t)%5 with LOCAL tile t; core c's features start at
        # c*IC, adding a class offset of (3*c*NT)%5 == (3c)%5 per core.
        w1p = np.stack([w1[(pidx + g + shift) % K] for g in range(K)], 0)
        b1p = np.stack([b1[(pidx + g + shift) % K] for g in range(K)], 0)
        w2p = np.stack([w2[(pidx + g + shift) % K] for g in range(K)], 0)
        b2p = np.stack([b2[(pidx + g + shift) % K] for g in range(K)], 0)
        pat16 = np.concatenate(
            [w1p.transpose(1, 0, 2).reshape(P, 5 * H),
             b1p.transpose(1, 0, 2).reshape(P, 5 * H),
             w2p.transpose(1, 0, 2).reshape(P, 5 * H)], axis=1).astype(bf16)
        cb32 = np.ascontiguousarray(b2p.T.astype(np.float32))  # [P, 5]
        return pat16, cb32

    x_bf = x.astype(bf16)
    in_maps = []
    for c in range(NCORES):
        sl = slice(c * IC, (c + 1) * IC)
        pat16, cb32 = patterns((3 * c * NT) % K)
        xt = np.ascontiguousarray(x_bf[:, sl].T)           # [IC, NB] bf16
        xs_sb = xt.reshape(NT, P, NB).transpose(1, 0, 2).reshape(P, NT * NB)
        cb16 = np.ascontiguousarray(
            np.concatenate([xs_sb, pat16], axis=1), dtype=bf16)
        V8 = W8_full[sl].reshape(NT, P, 2 * NO)
        wt8 = np.stack([
            np.ascontiguousarray(
                V8[q * NSUB:(q + 1) * NSUB].transpose(1, 0, 2)
                .reshape(P, NSUB * 2 * NO))
            for q in range(NCH)])                          # [NCH, P, 8*NO]
        in_maps.append({
            "cb16": cb16, "cb32": cb32,
            "Wt": np.ascontiguousarray(Wt16_full[sl]).view(bf16),
            "Wt8": wt8,
        })

    nc = _build()
    res = run_bass_kernel_spmd(nc, in_maps, list(range(NCORES)), trace=TRACE)
    LAST_RESULT = res
    out = np.zeros((B, O), dtype=np.float32)
    for c in range(NCORES):
        out += res.results[c]["out"]
    return out


# revision 31
# speedup vs baseline: 1.0265x; 1.0265x over previous
"""KAN layer kernel for 8 Trainium2 NeuronCores.

Math (reference):
    basis[b,i] = sum_h silu(x[b,i]*w1[i%K,h] + b1[i%K,h]) * w2[i%K,h] + b2[i%K]
    out[b,o]   = sum_i basis[b,i] * Wsum[o,i],   Wsum = W.sum(-1)   # [O,I]

Sharding: data-parallel over the input-feature axis I (16384 -> 8 x 2048).
Each core computes a partial out[64,1024] over its feature slice; host sums.

Per-core device program (memory-bound on reading its W slice):
  - W ships in mixed precision (rel err 1.1e-2 vs the 2e-2 gate): the k=0
    slice as fp8e4m3 in chunk-contiguous [chunk, p, (sub,o)] loads, k=1..4
    as bf16 rows [i, (k,o)] streaming tile-by-tile on the Sync queue; the
    final tile arrives in two k-slices so its matmuls overlap the DMA tail.
  - The k-reduction rides the PE: out[b,o] = sum_{i,k} basis[b,i]*W[o,i,k];
    each i-tile issues 5 (k) x 2 (O-half) matmuls reusing the same
    lhsT = basisT tile, accumulating all 80 into each PSUM bank.
  - basis is computed with i on partitions. DVE's bf16 2x mode needs every
    operand's last AP dim packed (stride 1): ACT materializes x broadcast
    over h (Copy), then all DVE tensor_tensor ops qualify. The h-reduction
    is a 2x-eligible add-tree; the last fold fuses +b2 and the bf16 cast in
    one scalar_tensor_tensor. ACT table thrash is avoided by pinning the
    silu_and_others table (serves Copy too) with a leading dummy silu.
  - w1/b1/w2/b2 patterns repeat every 5 i-tiles (128 % 5 = 3), so only 5
    pattern vectors ship instead of 16 per-tile copies.
"""
import numpy as np

B, I, O, K, H = 64, 16384, 1024, 5, 16
NCORES = 8
IC = I // NCORES          # 2048 features per core
P = 128                   # partition tile
NT = IC // P              # 16 i-tiles per core
NB = B                    # 64
NO = O                    # 1024
ROW = K * NO              # 5120 bf16 per Wt row
NSUB = 4                  # i-tiles per W chunk
NCH = NT // NSUB          # 4 chunks
# cb16 (bf16) column blocks: xs [NT*NB] | w1 [5*H] | b1 [5*H] | w2 [5*H]
X0 = NT * NB
X1 = X0 + 5 * H
X2 = X1 + 5 * H
CBW = X2 + 5 * H

TRACE = False             # test.py sets True to capture an NTFF profile
LAST_RESULT = None


def _build():
    from contextlib import ExitStack
    from concourse import bacc, mybir, tile

    f32 = mybir.dt.float32
    bf16 = mybir.dt.bfloat16
    nc = bacc.Bacc("TRN2", target_bir_lowering=False, debug=False,
                   num_devices=NCORES)
    f8 = mybir.dt.float8e4
    Wt = nc.declare_dram_parameter("Wt", [IC, 4 * NO], bf16, isOutput=False)
    Wt8 = nc.declare_dram_parameter("Wt8", [NCH, P, NSUB * NO], f8,
                                    isOutput=False)
    cb16d = nc.declare_dram_parameter("cb16", [P, CBW], bf16, isOutput=False)
    cb32d = nc.declare_dram_parameter("cb32", [P, 5], f32, isOutput=False)
    out = nc.declare_dram_parameter("out", [NB, NO], f32, isOutput=True)

    with tile.TileContext(nc) as tc, ExitStack() as ctx:
        const = ctx.enter_context(tc.tile_pool(name="const", bufs=1))
        wpool = ctx.enter_context(tc.tile_pool(name="w", bufs=1))
        ppool = ctx.enter_context(tc.tile_pool(name="pre", bufs=1))
        spool = ctx.enter_context(tc.tile_pool(name="silu", bufs=1))
        mpool = ctx.enter_context(tc.tile_pool(name="msum", bufs=1))
        apool = ctx.enter_context(tc.tile_pool(name="acc", bufs=NT))
        opool = ctx.enter_context(tc.tile_pool(name="out", bufs=1))
        psum = ctx.enter_context(tc.tile_pool(name="psum", bufs=1, space="PSUM"))

        # All bulk DMAs ride the Sync HWDGE queue: the Activation queue's
        # descriptor generation shares the ACT sequencer with Copy/Silu
        # compute and stalls badly. cb loads lead so compute starts right
        # after the fixed ~7.5us preamble; W streams tile-by-tile behind,
        # the last tile in two k-slices so its matmuls overlap the tail.
        cb16 = const.tile([P, CBW], bf16)
        cb32 = const.tile([P, 5], f32)
        nc.sync.dma_start(cb16[:, :], cb16d[:, :])
        nc.sync.dma_start(cb32[:, :], cb32d[:, :])

        ps0 = psum.tile([NB, 512], f32, tag="ps0")
        ps1 = psum.tile([NB, 512], f32, tag="ps1")

        # k=0 ships as fp8e4m3 (error headroom allows one of five slices),
        # chunk-contiguous so descriptors stay 4KB; k=1..4 stay bf16 and
        # stream tile-by-tile behind their chunk's fp8 load.
        wts, w8s = [], []
        for t in range(NT):
            if t % NSUB == 0:
                q = t // NSUB
                w8 = wpool.tile([P, NSUB * NO], f8, tag="w8", bufs=NCH)
                nc.sync.dma_start(w8[:, :], Wt8[q, :, :])
                w8s.append(w8)
            wt = wpool.tile([P, 4 * NO], bf16, tag="wt", bufs=6)
            if t == NT - 1:
                nc.sync.dma_start(wt[:, 0:2 * NO],
                                  Wt[t * P:(t + 1) * P, 0:2 * NO])
                nc.sync.dma_start(wt[:, 2 * NO:4 * NO],
                                  Wt[t * P:(t + 1) * P, 2 * NO:4 * NO])
            else:
                nc.sync.dma_start(wt[:, :], Wt[t * P:(t + 1) * P, :])
            wts.append(wt)

        def wk(t, k, lo, hi):
            # rhs slice for (tile, k): k=0 from the fp8 chunk, else bf16.
            if k == 0:
                q, s = divmod(t, NSUB)
                return w8s[q][:, s * NO + lo:s * NO + hi]
            return wts[t][:, (k - 1) * NO + lo:(k - 1) * NO + hi]

        # Pin the silu_and_others table (it also serves Copy) so the whole
        # kernel needs one ACT_TABLE_LOAD.
        dummy = const.tile([1, 1], f32)
        nc.scalar.activation(dummy[:, :], cb32[0:1, 0:1],
                             mybir.ActivationFunctionType.Silu)

        xreps, pre2s, accs = [], [], []

        def emit_front(t):
            # ACT: materialize x broadcast over h; DVE: 2x-mode affine.
            g = (3 * t) % 5
            xs = cb16[:, t * NB:(t + 1) * NB]              # [P, 64]
            w1s = cb16[:, X0 + g * H:X0 + (g + 1) * H]     # [P, 16]
            b1s = cb16[:, X1 + g * H:X1 + (g + 1) * H]
            xrep = ppool.tile([P, NB, H], bf16, tag="xrep", bufs=3)
            nc.scalar.copy(xrep[:, :, :],
                           xs[:, :, None].to_broadcast([P, NB, H]))
            xreps.append(xrep)
            pre = ppool.tile([P, NB, H], bf16, tag="pre", bufs=2)
            nc.vector.tensor_tensor(
                pre[:, :, :], xrep[:, :, :],
                w1s[:, None, :].to_broadcast([P, NB, H]),
                mybir.AluOpType.mult)
            pre2 = ppool.tile([P, NB, H], bf16, tag="pre2", bufs=3)
            nc.vector.tensor_tensor(
                pre2[:, :, :], pre[:, :, :],
                b1s[:, None, :].to_broadcast([P, NB, H]),
                mybir.AluOpType.add)
            pre2s.append(pre2)

        def emit_back(t):
            # ACT: silu; DVE: w2 mult + 2x add-tree + fused +b2/bf16 cast.
            g = (3 * t) % 5
            w2s = cb16[:, X2 + g * H:X2 + (g + 1) * H]
            s = spool.tile([P, NB, H], bf16, tag="s", bufs=3)
            nc.scalar.activation(s[:, :, :], pre2s[t][:, :, :],
                                 mybir.ActivationFunctionType.Silu)
            sw = spool.tile([P, NB, H], bf16, tag="sw", bufs=2)
            nc.vector.tensor_tensor(
                sw[:, :, :], s[:, :, :],
                w2s[:, None, :].to_broadcast([P, NB, H]),
                mybir.AluOpType.mult)
            f1 = mpool.tile([P, NB, H // 2], bf16, tag="f1", bufs=2)
            nc.vector.tensor_tensor(
                f1[:, :, :], sw[:, :, 0:H // 2], sw[:, :, H // 2:H],
                mybir.AluOpType.add)
            f2 = mpool.tile([P, NB, H // 4], bf16, tag="f2", bufs=2)
            nc.vector.tensor_tensor(
                f2[:, :, :], f1[:, :, 0:H // 4], f1[:, :, H // 4:H // 2],
                mybir.AluOpType.add)
            f3 = mpool.tile([P, NB, 2], bf16, tag="f3", bufs=2)
            nc.vector.tensor_tensor(
                f3[:, :, :], f2[:, :, 0:2], f2[:, :, 2:4],
                mybir.AluOpType.add)
            acc = apool.tile([P, NB], bf16, tag="acc")
            # acc = (f3[...,0] + b2) + f3[...,1], cast to bf16
            nc.vector.scalar_tensor_tensor(
                acc[:, :], f3[:, :, 0], cb32[:, g:g + 1], f3[:, :, 1],
                op0=mybir.AluOpType.add, op1=mybir.AluOpType.add)
            accs.append(acc)

        # Software-pipelined emission: the front half (copy + affine) of
        # tile t+1 is emitted before the back half of tile t, so neither
        # engine ever waits on the other's just-issued work.
        emit_front(0)
        for t in range(1, NT):
            emit_front(t)
            emit_back(t - 1)
        emit_back(NT - 1)

        # ---- partial matmuls: out[b,o] += sum_k basisT.T @ W[:,k,:] ----
        # Last tile runs all ps0 matmuls before ps1's so the ps0 bank can
        # drain (copy + store) while ps1 is still accumulating.
        for t in range(NT - 1):
            for k in range(K):
                first = (t == 0 and k == 0)
                nc.tensor.matmul(ps0[:, :], accs[t][:, :],
                                 wk(t, k, 0, 512),
                                 start=first, stop=False)
                nc.tensor.matmul(ps1[:, :], accs[t][:, :],
                                 wk(t, k, 512, NO),
                                 start=first, stop=False)
        tl = NT - 1
        out_sb = opool.tile([NB, NO], f32)
        for k in range(K):
            nc.tensor.matmul(ps0[:, :], accs[tl][:, :],
                             wk(tl, k, 0, 512),
                             start=False, stop=(k == K - 1))
        nc.vector.tensor_copy(out_sb[:, 0:512], ps0[:, :])
        nc.sync.dma_start(out[:, 0:512], out_sb[:, 0:512])
        for k in range(K):
            nc.tensor.matmul(ps1[:, :], accs[tl][:, :],
                             wk(tl, k, 512, NO),
                             start=False, stop=(k == K - 1))
        nc.vector.tensor_copy(out_sb[:, 512:1024], ps1[:, :])
        nc.sync.dma_start(out[:, 512:1024], out_sb[:, 512:1024])
    nc.compile()
    return nc


def kernel(x, w1, b1, w2, b2, W):
    global LAST_RESULT
    import ml_dtypes
    from concourse.bass_utils import run_bass_kernel_spmd

    bf16 = ml_dtypes.bfloat16
    x = np.asarray(x, dtype=np.float32)
    W = np.asarray(W, dtype=np.float32)
    w1 = np.asarray(w1, dtype=np.float32)
    b1 = np.asarray(b1, dtype=np.float32)
    w2 = np.asarray(w2, dtype=np.float32)
    b2 = np.asarray(b2, dtype=np.float32)

    # ---- host prep: k=1..4 -> bf16 rows [i, (k,o)]; k=0 -> fp8e4m3,
    # chunk-contiguous [chunk, p, (sub,o)] ----
    f8 = ml_dtypes.float8_e4m3fn
    Wb = W.astype(bf16).view(np.uint16)                    # [O, I, K]
    Wtr = np.ascontiguousarray(Wb.transpose(1, 2, 0))      # [I, K, O] u16
    Wt16_full = np.ascontiguousarray(Wtr[:, 1:, :]).reshape(I, 4 * NO)
    W8_full = np.ascontiguousarray(W[:, :, 0].T).astype(f8)  # [I, O]

    pidx = np.arange(P)

    def patterns(shift):
        # Device uses g=(3t)%5 with LOCAL tile t; core c's features start at
        # c*IC, adding a class offset of (3*c*NT)%5 == (3c)%5 per core.
        w1p = np.stack([w1[(pidx + g + shift) % K] for g in range(K)], 0)
        b1p = np.stack([b1[(pidx + g + shift) % K] for g in range(K)], 0)
        w2p = np.stack([w2[(pidx + g + shift) % K] for g in range(K)], 0)
        b2p = np.stack([b2[(pidx + g + shift) % K] for g in range(K)], 0)
        pat16 = np.concatenate(
            [w1p.transpose(1, 0, 2).reshape(P, 5 * H),
             b1p.transpose(1, 0, 2).reshape(P, 5 * H),
             w2p.transpose(1, 0, 2).reshape(P, 5 * H)], axis=1).astype(bf16)
        cb32 = np.ascontiguousarray(b2p.T.astype(np.float32))  # [P, 5]
        return pat16, cb32

    x_bf = x.astype(bf16)
    in_maps = []
    for c in range(NCORES):
        sl = slice(c * IC, (c + 1) * IC)
        pat16, cb32 = patterns((3 * c * NT) % K)
        xt = np.ascontiguousarray(x_bf[:, sl].T)           # [IC, NB] bf16
        xs_sb = xt.reshape(NT, P, NB).transpose(1, 0, 2).reshape(P, NT * NB)
        cb16 = np.ascontiguousarray(
            np.concatenate([xs_sb, pat16], axis=1), dtype=bf16)
        V8 = W8_full[sl].reshape(NT, P, NO)
        wt8 = np.stack([
            np.ascontiguousarray(
                V8[q * NSUB:(q + 1) * NSUB].transpose(1, 0, 2)
                .reshape(P, NSUB * NO))
            for q in range(NCH)])                          # [NCH, P, 4*NO]
        in_maps.append({
            "cb16": cb16, "cb32": cb32,
            "Wt": np.ascontiguousarray(Wt16_full[sl]).view(bf16),
            "Wt8": wt8,
        })

    nc = _build()
    res = run_bass_kernel_spmd(nc, in_maps, list(range(NCORES)), trace=TRACE)
    LAST_RESULT = res
    out = np.zeros((B, O), dtype=np.float32)
    for c in range(NCORES):
        out += res.results[c]["out"]
    return out
